# revision 58
# baseline (speedup 1.0000x reference)
"""ConvNMF loss kernel for Trainium2, sharded over 8 NeuronCores.

Math (see reference):
    W = softplus(W_pre)            # (F, K, L)
    H = softplus(H_pre)            # (K, T + L - 1)
    pred[f, t] = sum_{k,l} W[f,k,l] * H[k, T-1+l-t]
    out = sum((pred - data)^2) / (F*T)

Sharding: timebins split across 8 cores (sequence parallel), W replicated,
each core's H shard carries an L-1 halo. Per-core partial SSE is returned as
a [128,1] per-partition vector; the host does the final (tiny) reduction.

Device-side formulation: with Hs[k,j] = softplus(H_pre[k, T+30-t0-j]) (the
host flips each H shard in time), the core computes the forward correlation
    pred[f, t0+i] = sum_{k,l'} W[f,k,31-l'] * Hs[k, i+l']
as 16 accumulating 128-contraction matmuls per output tile: lag pairs
(2j, 2j+1) are packed into the 128 partitions (64 components x 2 lags). The
packed rhs Hdup[0:64,m]=Hs[:,m], Hdup[64:128,m]=Hs[:,m+1] is built host-side
from H_pre (pure reindexing; softplus runs on device).
"""
import numpy as np
from contextlib import ExitStack

import concourse.bass as bass
import concourse.bacc as bacc
import concourse.mybir as mybir
import concourse.tile as tile
from concourse import bass_utils

F32 = mybir.dt.float32
F32R = mybir.dt.float32r
AFT = mybir.ActivationFunctionType

F = 256           # n_features
K = 64            # n_components
L = 32            # n_lags
T = 20000         # n_timebins
NCORES = 8
TC = T // NCORES  # timebins per core (2500)
TW = 500          # output tile width (<=512 fp32 PSUM bank, >=256 for f32r rate)
NT = TC // TW     # 5 tiles per f-chunk
NJ = L // 2       # 16 lag pairs -> 16 matmuls per output tile
FCH = F // 128    # 2 f-chunks of 128
HC = TC + L - 1   # 2531 H columns per core (halo included)
NGROUPS = FCH * NT  # 10 PSUM tiles


FP8 = mybir.dt.float8e4
BF16 = mybir.dt.bfloat16
HPAD = 2544      # hq8 row stride, multiple of 16 (DoubleRow AP constraint)
NQ = L // 4      # 8 quad-lag DoubleRow matmuls per output tile


def build_nc(mode="fp8", reps=1, full_reps=False):
    if mode == "v2":
        return build_nc_v2(reps=reps, full_reps=full_reps)
    if mode == "v2safe":
        return build_nc_v2(reps=reps, full_reps=full_reps,
                           ln_fp8=False, inplace_ttr=False)
    if mode == "v2a":  # safe act, in-place ttr
        return build_nc_v2(reps=reps, full_reps=full_reps,
                           ln_fp8=False, inplace_ttr=True)
    if mode == "v2b":  # fp8 act out, no in-place ttr
        return build_nc_v2(reps=reps, full_reps=full_reps,
                           ln_fp8=True, inplace_ttr=False)
    if mode == "v3":
        return build_nc_v3(reps=reps, full_reps=full_reps)
    if mode == "v4":
        return build_nc_v4(reps=reps, full_reps=full_reps)
    if mode == "v4dve":
        return build_nc_v4(reps=reps, full_reps=full_reps, mul_on="vector")
    if mode == "v5":
        return build_nc_v5(reps=reps, full_reps=full_reps)
    if mode == "v5dve":
        return build_nc_v5(reps=reps, full_reps=full_reps, mul_on="vector")
    if mode == "v6":
        return build_nc_v6(reps=reps, full_reps=full_reps, mul_on="vector")
    if mode == "v6g":
        return build_nc_v6(reps=reps, full_reps=full_reps, mul_on="gpsimd")
    nc = bacc.Bacc("TRN2", target_bir_lowering=False, debug=False)
    hdup_d = nc.dram_tensor("hdup", [128, HC], F32, kind="ExternalInput").ap()
    wmat_d = nc.dram_tensor("wmat", [128, NJ * F], F32, kind="ExternalInput").ap()
    data_d = nc.dram_tensor("dat", [F, TC], F32, kind="ExternalInput").ap()
    out_d = nc.dram_tensor("out", [128, NGROUPS], F32, kind="ExternalOutput").ap()

    mmdt = F32R if mode == "f32r" else F32

    with tile.TileContext(nc) as tc, ExitStack() as ctx:
        cpool = ctx.enter_context(tc.tile_pool(name="cpool", bufs=1))
        dpool = ctx.enter_context(tc.tile_pool(name="dpool", bufs=NGROUPS))
        spool = ctx.enter_context(tc.tile_pool(name="spool", bufs=NGROUPS))
        ppool = ctx.enter_context(tc.tile_pool(name="ppool", bufs=4, space="PSUM"))
        apool = ctx.enter_context(tc.tile_pool(name="apool", bufs=2))

        HCH = 640
        WCH = 4
        h_edges = list(range(0, HC, HCH)) + [HC]
        acc0 = None

        n_outer = reps if full_reps else 1
        n_inner = 1 if full_reps else reps

        for outer in range(n_outer):
            # ---- H: chunked DMA -> exp -> ln(x+1) so the first matmuls can
            # start as soon as the first column chunk of softplus(H) lands ----
            hraw = cpool.tile([128, HC], F32, tag="hraw")
            hexp = cpool.tile([128, HC], F32, tag="hexp")
            hsp = cpool.tile([128, HC], mmdt, tag="hsp")
            # W is consumed interleaved with the first output tile's matmuls;
            # chunk it the same way (4 chunks of 4 lag pairs).
            wchunks = []
            for wc in range(NJ // WCH):
                wraw = cpool.tile([128, WCH * F], F32, tag=f"wraw{wc}",
                                  name=f"wraw{wc}_{outer}")
                wexp = cpool.tile([128, WCH * F], F32, tag=f"wexp{wc}",
                                  name=f"wexp{wc}_{outer}")
                w_t = cpool.tile([128, WCH * F], mmdt, tag=f"wsp{wc}",
                                 name=f"wsp{wc}_{outer}")
                wchunks.append((wraw, wexp, w_t))

            # interleave: H chunk 0, W chunk 0, H chunk 1, W chunks 1-3, rest of H
            def emit_h_chunk(i):
                lo, hi = h_edges[i], h_edges[i + 1]
                nc.sync.dma_start(hraw[:, lo:hi], hdup_d[:, lo:hi])
                nc.scalar.activation(hexp[:, lo:hi], hraw[:, lo:hi], AFT.Exp)
                nc.scalar.activation(hsp[:, lo:hi], hexp[:, lo:hi], AFT.Ln,
                                     bias=1.0)

            def emit_w_chunk(wc):
                wraw, wexp, w_t = wchunks[wc]
                nc.sync.dma_start(wraw[:], wmat_d[:, wc * WCH * F:(wc + 1) * WCH * F])
                nc.scalar.activation(wexp[:], wraw[:], AFT.Exp)
                nc.scalar.activation(w_t[:], wexp[:], AFT.Ln, bias=1.0)

            # fp8 path: DoubleRow matmuls contract 256 rows (2 subtiles); H and
            # W are converted f32->fp8e4 on DVE right after each softplus chunk.
            if mode == "fp8":
                hq8 = cpool.tile([128, 2, HPAD], FP8, tag="hq8")
                wq8 = [cpool.tile([128, WCH * F], FP8, tag=f"wq8{wc}",
                                  name=f"wq8{wc}_{outer}")
                       for wc in range(NJ // WCH)]

            def emit_h8_chunk(i):
                lo, hi = h_edges[i], h_edges[i + 1]
                nc.vector.tensor_copy(hq8[:, 0, lo:hi], hsp[:, lo:hi])
                m0 = max(0, lo - 2)
                nc.vector.tensor_copy(hq8[:, 1, m0:hi - 2], hsp[:, m0 + 2:hi])

            def emit_w8_chunk(wc):
                nc.vector.tensor_copy(wq8[wc][:], wchunks[wc][2][:])

            def post_h(i):
                if mode == "fp8":
                    emit_h8_chunk(i)

            def post_w(wc):
                if mode == "fp8":
                    emit_w8_chunk(wc)

            emit_h_chunk(0); post_h(0)
            emit_w_chunk(0); post_w(0)
            emit_h_chunk(1); post_h(1)
            emit_w_chunk(1); post_w(1)
            emit_w_chunk(2); post_w(2)
            emit_w_chunk(3); post_w(3)
            for i in range(2, len(h_edges) - 1):
                emit_h_chunk(i); post_h(i)

            def w_lhsT(j, c):
                w_t = wchunks[j // WCH][2]
                off = (j % WCH) * F + c * 128
                return w_t[:, off:off + 128]

            # ---- main loop: 10 output tiles ----
            for r in range(n_inner):
                rr = outer * n_inner + r
                acc = apool.tile([128, NGROUPS], F32, tag="acc", name=f"acc{rr}")
                if rr == 0:
                    acc0 = acc
                g = 0
                for c in range(FCH):
                    for i0 in range(NT):
                        pt = ppool.tile([128, TW], F32, tag="pt",
                                        name=f"pt{rr}_{g}")
                        base = i0 * TW
                        if mode == "fp8":
                            for jq in range(NQ):
                                wc, q = jq // 2, jq % 2
                                lhsT = wq8[wc][:, q * 512:(q + 1) * 512].rearrange(
                                    "p (i m) -> p i m", i=2)[:, :, c * 128:(c + 1) * 128]
                                rhs = hq8[:, :, base + 4 * jq: base + 4 * jq + TW]
                                nc.tensor.matmul(
                                    pt[:], lhsT, rhs, start=(jq == 0),
                                    stop=(jq == NQ - 1),
                                    perf_mode=mybir.MatmulPerfMode.DoubleRow)
                        else:
                            for j in range(NJ):
                                rhs = hsp[:, base + 2 * j: base + 2 * j + TW]
                                nc.tensor.matmul(pt[:], w_lhsT(j, c), rhs,
                                                 start=(j == 0), stop=(j == NJ - 1))

                        dt_ = dpool.tile([128, TW], F32, tag="dt",
                                         name=f"dt{rr}_{g}")
                        nc.sync.dma_start(
                            dt_[:], data_d[c * 128:(c + 1) * 128, base:base + TW])
                        resid = spool.tile([128, TW], F32, tag="resid",
                                           name=f"resid{rr}_{g}")
                        nc.vector.tensor_sub(resid[:], pt[:], dt_[:])
                        sq = spool.tile([128, TW], F32, tag="sq", name=f"sq{rr}_{g}")
                        nc.scalar.activation(sq[:], resid[:], AFT.Square,
                                             accum_out=acc[:, g:g + 1])
                        g += 1

        # ---- final: DMA the per-tile column sums out; host sums them ----
        nc.sync.dma_start(out_d[:], acc0[:])
    nc.compile()
    return nc


def build_nc_v2(reps=1, full_reps=False, ln_fp8=True, inplace_ttr=True):
    """v2: fp8 DoubleRow matmuls; single-pass Softplus with direct fp8 out
    (run twice for the two DoubleRow row groups of H); pred-data subtraction
    folded into the PE as one extra f32r matmul with a -identity stationary;
    square+accumulate on DVE via tensor_tensor_reduce. Engines: PE ~18us,
    Act ~11us, DVE ~7us, DMA ~17us -> PE-bound as target_regime demands."""
    nc = bacc.Bacc("TRN2", target_bir_lowering=False, debug=False)
    hdup_d = nc.dram_tensor("hdup", [128, HC], F32, kind="ExternalInput").ap()
    wmat_d = nc.dram_tensor("wmat", [128, NJ * F], F32, kind="ExternalInput").ap()
    data_d = nc.dram_tensor("dat", [F, TC], F32, kind="ExternalInput").ap()
    out_d = nc.dram_tensor("out", [128, NGROUPS], F32, kind="ExternalOutput").ap()

    with tile.TileContext(nc) as tc, ExitStack() as ctx:
        cpool = ctx.enter_context(tc.tile_pool(name="cpool", bufs=1))
        dpool = ctx.enter_context(tc.tile_pool(name="dpool", bufs=NGROUPS))
        ppool = ctx.enter_context(tc.tile_pool(name="ppool", bufs=4, space="PSUM"))
        apool = ctx.enter_context(tc.tile_pool(name="apool", bufs=2))

        HCH = 640
        WCH = 4
        h_edges = list(range(0, HC, HCH)) + [HC]
        nh = len(h_edges) - 1
        acc0 = None

        n_outer = reps if full_reps else 1
        n_inner = 1 if full_reps else reps

        for outer in range(n_outer):
            # one act table (natural_log_exp_and_others, id 6) serves Exp+Ln:
            # loading it explicitly stops the auto-pass thrashing 0<->5
            nc.scalar.add_instruction(mybir.InstLoadActFuncSet(
                name=nc.get_next_instruction_name(), act_func_set_id=6,
                ins=[], outs=[]))
            hraw = cpool.tile([128, HC], F32, tag="hraw")
            hexp = cpool.tile([128, HC], F32, tag="hexp")
            hq8 = cpool.tile([128, 2, HPAD], FP8, tag="hq8")
            wraw = [cpool.tile([128, WCH * F], F32, tag=f"wraw{wc}",
                               name=f"wraw{wc}_{outer}")
                    for wc in range(NJ // WCH)]
            wexp = [cpool.tile([128, WCH * F], F32, tag=f"wexp{wc}",
                               name=f"wexp{wc}_{outer}")
                    for wc in range(NJ // WCH)]
            wq8 = [cpool.tile([128, WCH * F], FP8, tag=f"wq8{wc}",
                              name=f"wq8{wc}_{outer}")
                   for wc in range(NJ // WCH)]

            hsp = (None if ln_fp8 else
                   cpool.tile([128, HC], F32, tag="hsp"))
            wsp = (None if ln_fp8 else
                   [cpool.tile([128, WCH * F], F32, tag=f"wsp{wc}",
                               name=f"wsp{wc}_{outer}")
                    for wc in range(NJ // WCH)])

            def emit_h_chunk(i):
                lo, hi = h_edges[i], h_edges[i + 1]
                nc.sync.dma_start(hraw[:, lo:hi], hdup_d[:, lo:hi])
                # softplus = ln(1+exp(x)); Ln writes fp8 row group 0 directly
                nc.scalar.activation(hexp[:, lo:hi], hraw[:, lo:hi], AFT.Exp)
                m0 = max(0, lo - 2)
                if ln_fp8:
                    nc.scalar.activation(hq8[:, 0, lo:hi], hexp[:, lo:hi],
                                         AFT.Ln, bias=1.0)
                    # row group 1: hq8[:,1,m] = hq8[:,0,m+2] (DVE fp8 copy);
                    # windows end 2 early so each copy reads its own chunk
                    nc.vector.tensor_copy(hq8[:, 1, m0:hi - 2],
                                          hq8[:, 0, m0 + 2:hi])
                else:
                    nc.scalar.activation(hsp[:, lo:hi], hexp[:, lo:hi],
                                         AFT.Ln, bias=1.0)
                    nc.vector.tensor_copy(hq8[:, 0, lo:hi], hsp[:, lo:hi])
                    nc.vector.tensor_copy(hq8[:, 1, m0:hi - 2],
                                          hsp[:, m0 + 2:hi])

            def emit_w_chunk(wc):
                nc.sync.dma_start(wraw[wc][:],
                                  wmat_d[:, wc * WCH * F:(wc + 1) * WCH * F])
                nc.scalar.activation(wexp[wc][:], wraw[wc][:], AFT.Exp)
                if ln_fp8:
                    nc.scalar.activation(wq8[wc][:], wexp[wc][:], AFT.Ln,
                                         bias=1.0)
                else:
                    nc.scalar.activation(wsp[wc][:], wexp[wc][:], AFT.Ln,
                                         bias=1.0)
                    nc.vector.tensor_copy(wq8[wc][:], wsp[wc][:])

            emit_h_chunk(0)
            emit_w_chunk(0)
            emit_h_chunk(1)
            emit_w_chunk(1)
            emit_w_chunk(2)
            emit_w_chunk(3)
            for i in range(2, nh):
                emit_h_chunk(i)

            for r in range(n_inner):
                rr = outer * n_inner + r
                acc = apool.tile([128, NGROUPS], F32, tag="acc", name=f"acc{rr}")
                if rr == 0:
                    acc0 = acc
                g = 0
                for c in range(FCH):
                    for i0 in range(NT):
                        pt = ppool.tile([128, TW], F32, tag="pt",
                                        name=f"pt{rr}_{g}")
                        base = i0 * TW
                        for jq in range(NQ):
                            wc, q = jq // 2, jq % 2
                            lhsT = wq8[wc][:, q * 512:(q + 1) * 512].rearrange(
                                "p (i m) -> p i m", i=2)[:, :, c * 128:(c + 1) * 128]
                            rhs = hq8[:, :, base + 4 * jq: base + 4 * jq + TW]
                            nc.tensor.matmul(
                                pt[:], lhsT, rhs, start=(jq == 0),
                                stop=(jq == NQ - 1),
                                perf_mode=mybir.MatmulPerfMode.DoubleRow)
                        # resid = pred - data: one DVE pass PSUM->SBUF bf16
                        dt_ = dpool.tile([128, TW], F32, tag="dt",
                                         name=f"dt{rr}_{g}")
                        nc.sync.dma_start(
                            dt_[:], data_d[c * 128:(c + 1) * 128, base:base + TW])
                        residb = dpool.tile([128, TW], BF16, tag="residb",
                                            name=f"residb{rr}_{g}")
                        nc.vector.tensor_sub(residb[:], pt[:], dt_[:])
                        # acc[:, g] = sum_t resid^2 via native DVE ops
                        # (TensorTensorReduce is a custom-ISA op that faults
                        # this runtime; mult+reduce in bf16 hit 2x/4x modes)
                        sq = dpool.tile([128, TW], BF16, tag="sq",
                                        name=f"sq{rr}_{g}")
                        nc.vector.tensor_mul(sq[:], residb[:], residb[:])
                        nc.vector.tensor_reduce(
                            acc[:, g:g + 1], sq[:], mybir.AxisListType.X,
                            mybir.AluOpType.add)
                        g += 1

        nc.sync.dma_start(out_d[:], acc0[:])
    nc.compile()
    return nc


def build_nc_v3(reps=1, full_reps=False):
    """v3: wave-major schedule. For each f-half c, the NQ=8 DoubleRow lag
    waves sweep all 5 time tiles before advancing, so the PE only ever waits
    on ONE W chunk (not all of W as tile-major does). W is regrouped on the
    host per (jq, c) so the c=0 pass streams just 1MB. All HBM transfers are
    bf16 (device still does softplus/matmuls/loss; bf16 transfer error is
    ~0.3% random, far inside the fp8 noise). Epilogue: DVE sub+mul+reduce
    (native ops only)."""
    nc = bacc.Bacc("TRN2", target_bir_lowering=False, debug=False)
    hdup_d = nc.dram_tensor("hdupb", [128, HC], BF16, kind="ExternalInput").ap()
    wmat_d = nc.dram_tensor("wmatb", [128, NJ * F], BF16, kind="ExternalInput").ap()
    data_d = nc.dram_tensor("datb", [F, TC], BF16, kind="ExternalInput").ap()
    out_d = nc.dram_tensor("out", [128, NGROUPS], F32, kind="ExternalOutput").ap()

    with tile.TileContext(nc) as tc, ExitStack() as ctx:
        cpool = ctx.enter_context(tc.tile_pool(name="cpool", bufs=1))
        dpool = ctx.enter_context(tc.tile_pool(name="dpool", bufs=NGROUPS))
        spool = ctx.enter_context(tc.tile_pool(name="spool", bufs=4))
        ppool = ctx.enter_context(tc.tile_pool(name="ppool", bufs=7, space="PSUM"))
        apool = ctx.enter_context(tc.tile_pool(name="apool", bufs=2))

        HCH = 640
        h_edges = list(range(0, HC, HCH)) + [HC]
        nh = len(h_edges) - 1
        acc0 = None

        n_outer = reps if full_reps else 1
        n_inner = 1 if full_reps else reps

        for outer in range(n_outer):
            nc.scalar.add_instruction(mybir.InstLoadActFuncSet(
                name=nc.get_next_instruction_name(), act_func_set_id=6,
                ins=[], outs=[]))
            hraw = cpool.tile([128, HC], BF16, tag="hraw")
            hexp = cpool.tile([128, HC], F32, tag="hexp")
            hq8 = cpool.tile([128, 2, HPAD], FP8, tag="hq8")
            # W chunks: one per (c, wc) = 2 jq groups = 512 cols
            wraw = {}
            wexp = {}
            wq8 = {}
            for c in range(FCH):
                for wc in range(NQ // 2):
                    wraw[c, wc] = cpool.tile([128, 512], BF16,
                                             tag=f"wraw{c}_{wc}",
                                             name=f"wraw{c}_{wc}_{outer}")
                    wexp[c, wc] = cpool.tile([128, 512], F32,
                                             tag=f"wexp{c}_{wc}",
                                             name=f"wexp{c}_{wc}_{outer}")
                    wq8[c, wc] = cpool.tile([128, 512], FP8,
                                            tag=f"wq8{c}_{wc}",
                                            name=f"wq8{c}_{wc}_{outer}")

            def emit_h_chunk(i):
                lo, hi = h_edges[i], h_edges[i + 1]
                nc.sync.dma_start(hraw[:, lo:hi], hdup_d[:, lo:hi])
                nc.scalar.activation(hexp[:, lo:hi], hraw[:, lo:hi], AFT.Exp)
                nc.scalar.activation(hq8[:, 0, lo:hi], hexp[:, lo:hi],
                                     AFT.Ln, bias=1.0)
                m0 = max(0, lo - 2)
                nc.vector.tensor_copy(hq8[:, 1, m0:hi - 2],
                                      hq8[:, 0, m0 + 2:hi])

            def emit_w_chunk(c, wc):
                # host layout (c, jq, i, fo): chunk (c, wc) = jq {2wc, 2wc+1}
                off = c * 2048 + wc * 512
                nc.sync.dma_start(wraw[c, wc][:], wmat_d[:, off:off + 512])
                nc.scalar.activation(wexp[c, wc][:], wraw[c, wc][:], AFT.Exp)
                nc.scalar.activation(wq8[c, wc][:], wexp[c, wc][:],
                                     AFT.Ln, bias=1.0)

            # DMA/act order: H feeds the whole first wave, so H leads;
            # c=0 W chunks next, c=1 W chunks stream during the c=0 pass.
            emit_h_chunk(0)
            emit_w_chunk(0, 0)
            emit_h_chunk(1)
            emit_w_chunk(0, 1)
            emit_h_chunk(2)
            emit_w_chunk(0, 2)
            emit_h_chunk(3)
            emit_w_chunk(0, 3)
            for i in range(4, nh):
                emit_h_chunk(i)
            for wc in range(NQ // 2):
                emit_w_chunk(1, wc)

            for r in range(n_inner):
                rr = outer * n_inner + r
                acc = apool.tile([128, NGROUPS], F32, tag="acc", name=f"acc{rr}")
                if rr == 0:
                    acc0 = acc
                for c in range(FCH):
                    pts = [ppool.tile([128, TW], F32, tag="pt",
                                      name=f"pt{rr}_{c}_{i0}")
                           for i0 in range(NT)]
                    # data DMA early (cheap, overlaps waves)
                    dtbs = []
                    for i0 in range(NT):
                        base = i0 * TW
                        dtb = dpool.tile([128, TW], BF16, tag="dtb",
                                         name=f"dtb{rr}_{c}_{i0}")
                        nc.sync.dma_start(
                            dtb[:],
                            data_d[c * 128:(c + 1) * 128, base:base + TW])
                        dtbs.append(dtb)
                    for jq in range(NQ):
                        wc, q = jq // 2, jq % 2
                        lhsT = wq8[c, wc][:, q * 256:(q + 1) * 256].rearrange(
                            "p (i m) -> p i m", i=2)
                        for i0 in range(NT):
                            base = i0 * TW
                            rhs = hq8[:, :, base + 4 * jq: base + 4 * jq + TW]
                            nc.tensor.matmul(
                                pts[i0][:], lhsT, rhs, start=(jq == 0),
                                stop=(jq == NQ - 1),
                                perf_mode=mybir.MatmulPerfMode.DoubleRow)
                    for i0 in range(NT):
                        g = c * NT + i0
                        residb = spool.tile([128, TW], BF16, tag="residb",
                                            name=f"residb{rr}_{g}")
                        nc.vector.tensor_sub(residb[:], pts[i0][:], dtbs[i0][:])
                        sq = spool.tile([128, TW], BF16, tag="sq",
                                        name=f"sq{rr}_{g}")
                        nc.vector.tensor_mul(sq[:], residb[:], residb[:])
                        nc.vector.tensor_reduce(
                            acc[:, g:g + 1], sq[:], mybir.AxisListType.X,
                            mybir.AluOpType.add)

        nc.sync.dma_start(out_d[:], acc0[:])
    nc.compile()
    return nc


def build_nc_v4(reps=1, full_reps=False, mul_on="gpsimd", red_on="vector"):
    """v4: tile-major (staggered tile completion -> epilogues overlap the
    main loop) with W softplus prioritized per f-half so tile (c,0) closes
    early. bf16 HBM transfers; Act does only softplus (coarse 2048-col W
    ops); per-tile epilogue split across engines:
    DVE sub -> [mul_on] residb^2 -> [red_on] column reduce."""
    nc = bacc.Bacc("TRN2", target_bir_lowering=False, debug=False)
    hdup_d = nc.dram_tensor("hdupb", [128, HC], BF16, kind="ExternalInput").ap()
    wmat_d = nc.dram_tensor("wmatb", [128, NJ * F], BF16, kind="ExternalInput").ap()
    data_d = nc.dram_tensor("datb", [F, TC], BF16, kind="ExternalInput").ap()
    out_d = nc.dram_tensor("out", [128, NGROUPS], F32, kind="ExternalOutput").ap()

    mul_eng = nc.gpsimd if mul_on == "gpsimd" else nc.vector
    red_eng = nc.gpsimd if red_on == "gpsimd" else nc.vector

    with tile.TileContext(nc) as tc, ExitStack() as ctx:
        cpool = ctx.enter_context(tc.tile_pool(name="cpool", bufs=1))
        dpool = ctx.enter_context(tc.tile_pool(name="dpool", bufs=6))
        spool = ctx.enter_context(tc.tile_pool(name="spool", bufs=4))
        ppool = ctx.enter_context(tc.tile_pool(name="ppool", bufs=6, space="PSUM"))
        apool = ctx.enter_context(tc.tile_pool(name="apool", bufs=2))

        HCH = 640
        h_edges = list(range(0, HC, HCH)) + [HC]
        nh = len(h_edges) - 1
        acc0 = None

        n_outer = reps if full_reps else 1
        n_inner = 1 if full_reps else reps

        for outer in range(n_outer):
            nc.scalar.add_instruction(mybir.InstLoadActFuncSet(
                name=nc.get_next_instruction_name(), act_func_set_id=6,
                ins=[], outs=[]))
            hraw = cpool.tile([128, HC], BF16, tag="hraw")
            hexp = cpool.tile([128, HC], F32, tag="hexp")
            hq8 = cpool.tile([128, 2, HPAD], FP8, tag="hq8")
            # W per f-half c: raw + exp + fp8, softplus'd as one 2048-col op
            wraw = [cpool.tile([128, 2048], BF16, tag=f"wraw{c}",
                               name=f"wraw{c}_{outer}") for c in range(FCH)]
            wexp = [cpool.tile([128, 2048], F32, tag=f"wexp{c}",
                               name=f"wexp{c}_{outer}") for c in range(FCH)]
            wq8 = [cpool.tile([128, 2048], FP8, tag=f"wq8{c}",
                              name=f"wq8{c}_{outer}") for c in range(FCH)]

            def emit_h_chunk(i):
                lo, hi = h_edges[i], h_edges[i + 1]
                nc.sync.dma_start(hraw[:, lo:hi], hdup_d[:, lo:hi])
                nc.scalar.activation(hexp[:, lo:hi], hraw[:, lo:hi], AFT.Exp)
                nc.scalar.activation(hq8[:, 0, lo:hi], hexp[:, lo:hi],
                                     AFT.Ln, bias=1.0)
                m0 = max(0, lo - 2)
                nc.vector.tensor_copy(hq8[:, 1, m0:hi - 2],
                                      hq8[:, 0, m0 + 2:hi])

            def emit_w_half(c):
                # host layout groups (c, wc, q, i, fo): c half contiguous
                nc.sync.dma_start(wraw[c][:], wmat_d[:, c * 2048:(c + 1) * 2048])
                nc.scalar.activation(wexp[c][:], wraw[c][:], AFT.Exp)
                nc.scalar.activation(wq8[c][:], wexp[c][:], AFT.Ln, bias=1.0)

            # H chunk 0 first (tile (0,0) needs it), then all of W-c0 so the
            # c=0 tiles close early; remaining H; W-c1 last.
            emit_h_chunk(0)
            emit_w_half(0)
            for i in range(1, nh):
                emit_h_chunk(i)
            emit_w_half(1)

            for r in range(n_inner):
                rr = outer * n_inner + r
                acc = apool.tile([128, NGROUPS], F32, tag="acc", name=f"acc{rr}")
                if rr == 0:
                    acc0 = acc
                for c in range(FCH):
                    for i0 in range(NT):
                        g = c * NT + i0
                        base = i0 * TW
                        pt = ppool.tile([128, TW], F32, tag="pt",
                                        name=f"pt{rr}_{g}")
                        dtb = dpool.tile([128, TW], BF16, tag="dtb",
                                         name=f"dtb{rr}_{g}")
                        nc.sync.dma_start(
                            dtb[:],
                            data_d[c * 128:(c + 1) * 128, base:base + TW])
                        for jq in range(NQ):
                            wc, q = jq // 2, jq % 2
                            lhsT = wq8[c][:, jq * 256:(jq + 1) * 256].rearrange(
                                "p (i m) -> p i m", i=2)
                            rhs = hq8[:, :, base + 4 * jq: base + 4 * jq + TW]
                            nc.tensor.matmul(
                                pt[:], lhsT, rhs, start=(jq == 0),
                                stop=(jq == NQ - 1),
                                perf_mode=mybir.MatmulPerfMode.DoubleRow)
                        residb = spool.tile([128, TW], BF16, tag="residb",
                                            name=f"residb{rr}_{g}")
                        nc.vector.tensor_sub(residb[:], pt[:], dtb[:])
                        sq = spool.tile([128, TW], BF16, tag="sq",
                                        name=f"sq{rr}_{g}")
                        mul_eng.tensor_mul(sq[:], residb[:], residb[:])
                        red_eng.tensor_reduce(
                            acc[:, g:g + 1], sq[:], mybir.AxisListType.X,
                            mybir.AluOpType.add)

        nc.sync.dma_start(out_d[:], acc0[:])
    nc.compile()
    return nc


def build_nc_v5(reps=1, full_reps=False, mul_on="gpsimd"):
    """v5 = v4 with (a) W softplus in 512-col chunks interleaved with H so
    the first tile closes ~5us in, and (b) the SSE reduction done on the PE:
    per tile, resid^2 (bf16) is summed over partitions by a ones-vector
    matmul accumulating into a single [1,TW] PSUM bank across all 10 tiles;
    one final 500-col DVE reduce collapses it to a scalar. Epilogue per tile
    is then just DVE sub + gpsimd mul + a 208ns PE matmul, which drains
    behind production instead of serializing after it."""
    nc = bacc.Bacc("TRN2", target_bir_lowering=False, debug=False)
    hdup_d = nc.dram_tensor("hdupb", [128, HC], BF16, kind="ExternalInput").ap()
    wmat_d = nc.dram_tensor("wmatb", [128, NJ * F], BF16, kind="ExternalInput").ap()
    data_d = nc.dram_tensor("datb", [F, TC], BF16, kind="ExternalInput").ap()
    out_d = nc.dram_tensor("out", [1, 1], F32, kind="ExternalOutput").ap()

    mul_eng = nc.gpsimd if mul_on == "gpsimd" else nc.vector

    with tile.TileContext(nc) as tc, ExitStack() as ctx:
        cpool = ctx.enter_context(tc.tile_pool(name="cpool", bufs=1))
        dpool = ctx.enter_context(tc.tile_pool(name="dpool", bufs=6))
        spool = ctx.enter_context(tc.tile_pool(name="spool", bufs=4))
        ppool = ctx.enter_context(tc.tile_pool(name="ppool", bufs=6, space="PSUM"))
        qpool = ctx.enter_context(tc.tile_pool(name="qpool", bufs=1, space="PSUM"))
        apool = ctx.enter_context(tc.tile_pool(name="apool", bufs=2))

        # chunk 0 covers tile 0's full span (<=533); chunk edges chosen so
        # each later tile is gated by a distinct chunk (t3<=2033, t4<=2530)
        h_edges = [0, 640, 1280, 2048, HC]
        nh = len(h_edges) - 1
        out_sb0 = None

        n_outer = reps if full_reps else 1
        n_inner = 1 if full_reps else reps

        for outer in range(n_outer):
            nc.scalar.add_instruction(mybir.InstLoadActFuncSet(
                name=nc.get_next_instruction_name(), act_func_set_id=6,
                ins=[], outs=[]))
            hraw = cpool.tile([128, HC], BF16, tag="hraw")
            hexp = cpool.tile([128, HC], F32, tag="hexp")
            hq8 = cpool.tile([128, 2, HPAD], FP8, tag="hq8")
            ones = cpool.tile([128, 1], BF16, tag="ones")
            nc.vector.memset(ones[:], 1.0)
            # W-c0 in four 512 chunks (wave pacing); W-c1 one 2048-col op
            wraw = [cpool.tile([128, 2048], BF16, tag=f"wraw{c}",
                               name=f"wraw{c}_{outer}") for c in range(FCH)]
            wexp = [cpool.tile([128, 2048], F32, tag=f"wexp{c}",
                               name=f"wexp{c}_{outer}") for c in range(FCH)]
            wq8 = [cpool.tile([128, 2048], FP8, tag=f"wq8{c}",
                              name=f"wq8{c}_{outer}") for c in range(FCH)]

            def emit_h_chunk(i):
                lo, hi = h_edges[i], h_edges[i + 1]
                nc.sync.dma_start(hraw[:, lo:hi], hdup_d[:, lo:hi])
                nc.scalar.activation(hexp[:, lo:hi], hraw[:, lo:hi], AFT.Exp)
                nc.scalar.activation(hq8[:, 0, lo:hi], hexp[:, lo:hi],
                                     AFT.Ln, bias=1.0)
                m0 = max(0, lo - 2)
                nc.vector.tensor_copy(hq8[:, 1, m0:hi - 2],
                                      hq8[:, 0, m0 + 2:hi])

            def emit_w_cols(c, lo, hi):
                nc.sync.dma_start(wraw[c][:, lo:hi],
                                  wmat_d[:, c * 2048 + lo:c * 2048 + hi])
                nc.scalar.activation(wexp[c][:, lo:hi], wraw[c][:, lo:hi],
                                     AFT.Exp)
                nc.scalar.activation(wq8[c][:, lo:hi], wexp[c][:, lo:hi],
                                     AFT.Ln, bias=1.0)

            # act/DMA order matched to the PE tile order below: W-c0 early,
            # W-c1 split so c1 tiles can start half-waves while h2+ stream
            emit_h_chunk(0)
            emit_w_cols(0, 0, 256)      # jq 0 for the first matmul
            emit_w_cols(0, 256, 1024)   # jq 1-3
            emit_w_cols(0, 1024, 2048)  # jq 4-7
            emit_h_chunk(1)
            emit_w_cols(1, 0, 1024)     # c1 jq 0-3
            emit_h_chunk(2)
            emit_w_cols(1, 1024, 2048)  # c1 jq 4-7
            emit_h_chunk(3)

            # PE steps (c, i0, jq_lo, jq_hi): c1 tiles split into half-waves
            # that slot into the gaps while W-c1/h3 softplus completes
            steps = [
                (0, 0, 0, 8), (0, 1, 0, 8),
                (1, 0, 0, 4), (1, 1, 0, 4), (1, 2, 0, 4),
                (0, 2, 0, 8),
                (1, 0, 4, 8), (1, 1, 4, 8),
                (0, 3, 0, 8),
                (1, 2, 4, 8), (1, 3, 0, 8),
                (0, 4, 0, 8), (1, 4, 0, 8),
            ]

            for r in range(n_inner):
                rr = outer * n_inner + r
                accp = qpool.tile([1, TW], F32, tag="accp", name=f"accp{rr}")
                sq_mms = []  # deferred (ones @ sq) accumulation matmuls
                pts = {}
                dtbs = {}
                nclosed = 0
                for c, i0, jlo, jhi in steps:
                    g = c * NT + i0
                    base = i0 * TW
                    if jlo == 0:
                        pts[g] = ppool.tile([128, TW], F32, tag="pt",
                                            name=f"pt{rr}_{g}")
                        dtb = dpool.tile([128, TW], BF16, tag="dtb",
                                         name=f"dtb{rr}_{g}")
                        nc.sync.dma_start(
                            dtb[:],
                            data_d[c * 128:(c + 1) * 128, base:base + TW])
                        dtbs[g] = dtb
                    pt = pts[g]
                    for jq in range(jlo, jhi):
                        lhsT = wq8[c][:, jq * 256:(jq + 1) * 256].rearrange(
                            "p (i m) -> p i m", i=2)
                        rhs = hq8[:, :, base + 4 * jq: base + 4 * jq + TW]
                        nc.tensor.matmul(
                            pt[:], lhsT, rhs, start=(jq == 0),
                            stop=(jq == NQ - 1),
                            perf_mode=mybir.MatmulPerfMode.DoubleRow)
                    if jhi < NQ:
                        continue
                    # tile complete: epilogue; sq matmuls trail 3 tiles so
                    # the PE never waits on the DVE chain
                    nclosed += 1
                    if len(sq_mms) >= 3:
                        sq_mms.pop(0)()
                    residb = spool.tile([128, TW], BF16, tag="residb",
                                        name=f"residb{rr}_{g}")
                    nc.vector.tensor_sub(residb[:], pt[:], dtbs[g][:])
                    sq = spool.tile([128, TW], BF16, tag="sq",
                                    name=f"sq{rr}_{g}")
                    mul_eng.tensor_mul(sq[:], residb[:], residb[:])

                    def mk(sq=sq, n=nclosed):
                        def emit():
                            nc.tensor.matmul(accp[:], ones[:], sq[:],
                                             start=(n == 1),
                                             stop=(n == NGROUPS))
                        return emit
                    sq_mms.append(mk())
                for f_ in sq_mms:
                    f_()
                # collapse [1,TW] -> scalar and ship it out
                out_sb = apool.tile([1, 1], F32, tag="osb", name=f"osb{rr}")
                nc.vector.tensor_reduce(out_sb[:], accp[:],
                                        mybir.AxisListType.X,
                                        mybir.AluOpType.add)
                if rr == 0:
                    out_sb0 = out_sb

        nc.sync.dma_start(out_d[:], out_sb0[:])
    nc.compile()
    return nc


HPK = 1266  # packed-H columns: 64 rows x 2531 cols as [128, 1266]
HLD = 640   # v6 lead: first HLD cols arrive host-duplicated (ramp path)
HPK2 = (HC - HLD + 1) // 2  # 946: packed tail covers cols [HLD, HC)


def build_nc_v6(reps=1, full_reps=False, mul_on="gpsimd"):
    """v6 = v5 with packed-H softplus: the host sends H's 64 unique rows as
    [128, 1266] bf16 (two time-halves stacked in the partition dim), so the
    Act engine runs exp+ln over 1266 cols instead of 2531. Four SBUF->SBUF
    DMA moves + the usual DVE shift-copy assemble the duplicated DoubleRow
    hq8 layout from the packed fp8 softplus output."""
    nc = bacc.Bacc("TRN2", target_bir_lowering=False, debug=False)
    hld_d = nc.dram_tensor("hleadb", [128, HLD], BF16, kind="ExternalInput").ap()
    hpk_d = nc.dram_tensor("hpk2b", [128, HPK2], BF16, kind="ExternalInput").ap()
    wmat_d = nc.dram_tensor("wmatb", [128, NJ * F], BF16, kind="ExternalInput").ap()
    data_d = nc.dram_tensor("datb", [F, TC], BF16, kind="ExternalInput").ap()
    out_d = nc.dram_tensor("out", [1, 1], F32, kind="ExternalOutput").ap()

    mul_eng = nc.gpsimd if mul_on == "gpsimd" else nc.vector

    with tile.TileContext(nc) as tc, ExitStack() as ctx:
        cpool = ctx.enter_context(tc.tile_pool(name="cpool", bufs=1))
        dpool = ctx.enter_context(tc.tile_pool(name="dpool", bufs=6))
        spool = ctx.enter_context(tc.tile_pool(name="spool", bufs=4))
        ppool = ctx.enter_context(tc.tile_pool(name="ppool", bufs=6, space="PSUM"))
        qpool = ctx.enter_context(tc.tile_pool(name="qpool", bufs=1, space="PSUM"))
        apool = ctx.enter_context(tc.tile_pool(name="apool", bufs=2))

        out_sb0 = None

        n_outer = reps if full_reps else 1
        n_inner = 1 if full_reps else reps

        for outer in range(n_outer):
            nc.scalar.add_instruction(mybir.InstLoadActFuncSet(
                name=nc.get_next_instruction_name(), act_func_set_id=6,
                ins=[], outs=[]))
            hlraw = cpool.tile([128, HLD], BF16, tag="hlraw")
            hlexp = cpool.tile([128, HLD], F32, tag="hlexp")
            hraw = cpool.tile([128, HPK2], BF16, tag="hraw")
            hexp = cpool.tile([128, HPK2], F32, tag="hexp")
            sp8 = cpool.tile([128, HPK2], FP8, tag="sp8")
            hq8 = cpool.tile([128, 2, HPAD], FP8, tag="hq8")
            ones = cpool.tile([128, 1], BF16, tag="ones")
            nc.vector.memset(ones[:], 1.0)
            wraw = [cpool.tile([128, 2048], BF16, tag=f"wraw{c}",
                               name=f"wraw{c}_{outer}") for c in range(FCH)]
            wexp = [cpool.tile([128, 2048], F32, tag=f"wexp{c}",
                               name=f"wexp{c}_{outer}") for c in range(FCH)]
            wq8 = [cpool.tile([128, 2048], FP8, tag=f"wq8{c}",
                              name=f"wq8{c}_{outer}") for c in range(FCH)]

            def emit_h_lead():
                # first HLD cols arrive host-duplicated: softplus lands in
                # hq8 row 0 directly, one DVE copy makes the +2-shifted row
                # group; nothing else gates the first tile
                nc.sync.dma_start(hlraw[:], hld_d)
                nc.scalar.activation(hlexp[:], hlraw[:], AFT.Exp)
                nc.scalar.activation(hq8[:, 0, 0:HLD], hlexp[:],
                                     AFT.Ln, bias=1.0)
                nc.vector.tensor_copy(hq8[:, 1, 0:HLD - 2],
                                      hq8[:, 0, 2:HLD])

            def emit_h_packed_acts():
                # cols >= HLD come packed: rows 0:64 hold Hs[k, 640:1586],
                # rows 64:128 hold Hs[k, 1586:2531] (+1 pad col)
                nc.sync.dma_start(hraw[:], hpk_d)
                nc.scalar.activation(hexp[:], hraw[:], AFT.Exp)
                nc.scalar.activation(sp8[:], hexp[:], AFT.Ln, bias=1.0)

            def emit_h_moves():
                # 8 SBUF->SBUF DMAs assemble both hq8 row groups from sp8;
                # needed only from tile t1 on, so they can trail the W input
                # stream on the SP queue
                s, e = HLD, HLD + HPK2          # 640..1586
                v = HPK2 - 1                    # valid second-half cols
                nc.sync.dma_start(hq8[0:64, 0, s:e], sp8[0:64, :])
                nc.sync.dma_start(hq8[64:128, 0, s:e - 1], sp8[0:64, 1:HPK2])
                nc.sync.dma_start(hq8[0:64, 1, s - 2:e - 2], sp8[0:64, :])
                nc.sync.dma_start(hq8[64:128, 1, s - 2:e - 3],
                                  sp8[0:64, 1:HPK2])
                nc.sync.dma_start(hq8[0:64, 0, e:HC], sp8[64:128, 0:v])
                nc.sync.dma_start(hq8[64:128, 0, e - 1:HC - 1],
                                  sp8[64:128, 0:v])
                nc.sync.dma_start(hq8[0:64, 1, e - 2:HC - 2],
                                  sp8[64:128, 0:v])
                nc.sync.dma_start(hq8[64:128, 1, e - 3:HC - 3],
                                  sp8[64:128, 0:v])

            def emit_w_cols(c, lo, hi):
                nc.sync.dma_start(wraw[c][:, lo:hi],
                                  wmat_d[:, c * 2048 + lo:c * 2048 + hi])
                nc.scalar.activation(wexp[c][:, lo:hi], wraw[c][:, lo:hi],
                                     AFT.Exp)
                nc.scalar.activation(wq8[c][:, lo:hi], wexp[c][:, lo:hi],
                                     AFT.Ln, bias=1.0)

            # hpk2 softplus position among the W chunks is tunable (V6_PPOS)
            ppos = globals().get("V6_PPOS", 1)
            wchunks = [lambda: emit_w_cols(0, 0, 256),
                       lambda: emit_w_cols(0, 256, 1024),
                       lambda: emit_w_cols(0, 1024, 2048),
                       lambda: emit_w_cols(1, 0, 1024),
                       lambda: emit_w_cols(1, 1024, 2048)]
            emit_h_lead()
            for i, wemit in enumerate(wchunks):
                wemit()
                if i + 1 == ppos:
                    emit_h_packed_acts()
                    emit_h_moves()

            steps = globals().get("V6_STEPS") or [
                (0, 0, 0, 8), (0, 1, 0, 8), (0, 2, 0, 8),
                (1, 0, 0, 4), (0, 3, 0, 8),
                (1, 1, 0, 4), (0, 4, 0, 8),
                (1, 0, 4, 8), (1, 1, 4, 8),
                (1, 2, 0, 8), (1, 3, 0, 8), (1, 4, 0, 8),
            ]

            for r in range(n_inner):
                rr = outer * n_inner + r
                accp = qpool.tile([1, TW], F32, tag="accp", name=f"accp{rr}")
                sq_mms = []
                pts = {}
                dtbs = {}
                nclosed = 0
                for c, i0, jlo, jhi in steps:
                    g = c * NT + i0
                    base = i0 * TW
                    if jlo == 0:
                        pts[g] = ppool.tile([128, TW], F32, tag="pt",
                                            name=f"pt{rr}_{g}")
                        dtb = dpool.tile([128, TW], BF16, tag="dtb",
                                         name=f"dtb{rr}_{g}")
                        # data tiles ride the Pool queue: SWDGE is a separate
                        # descriptor-gen lane, parallel to the HWDGE the
                        # H/W input DMAs keep saturated
                        nc.gpsimd.dma_start(
                            dtb[:],
                            data_d[c * 128:(c + 1) * 128, base:base + TW])
                        dtbs[g] = dtb
                    pt = pts[g]
                    for jq in range(jlo, jhi):
                        lhsT = wq8[c][:, jq * 256:(jq + 1) * 256].rearrange(
                            "p (i m) -> p i m", i=2)
                        rhs = hq8[:, :, base + 4 * jq: base + 4 * jq + TW]
                        nc.tensor.matmul(
                            pt[:], lhsT, rhs, start=(jq == 0),
                            stop=(jq == NQ - 1),
                            perf_mode=mybir.MatmulPerfMode.DoubleRow)
                    if jhi < NQ:
                        continue
                    nclosed += 1
                    if len(sq_mms) >= 3:
                        sq_mms.pop(0)()
                    residb = spool.tile([128, TW], BF16, tag="residb",
                                        name=f"residb{rr}_{g}")
                    nc.vector.tensor_sub(residb[:], pt[:], dtbs[g][:])
                    sq = spool.tile([128, TW], BF16, tag="sq",
                                    name=f"sq{rr}_{g}")
                    mul_eng.tensor_mul(sq[:], residb[:], residb[:])

                    def mk(sq=sq, n=nclosed):
                        def emit():
                            nc.tensor.matmul(accp[:], ones[:], sq[:],
                                             start=(n == 1),
                                             stop=(n == NGROUPS))
                        return emit
                    sq_mms.append(mk())
                for f_ in sq_mms:
                    f_()
                out_sb = apool.tile([1, 1], F32, tag="osb", name=f"osb{rr}")
                nc.vector.tensor_reduce(out_sb[:], accp[:],
                                        mybir.AxisListType.X,
                                        mybir.AluOpType.add)
                if rr == 0:
                    out_sb0 = out_sb

        nc.sync.dma_start(out_d[:], out_sb0[:])
    nc.compile()
    return nc


def make_in_maps(data, W_pre, H_pre):
    """Pure host-side resharding/reindexing (no math beyond indexing)."""
    data = np.ascontiguousarray(data, dtype=np.float32)
    W_pre = np.asarray(W_pre, dtype=np.float32)
    H_pre = np.asarray(H_pre, dtype=np.float32)

    # W: [128, 16*256]; rows (l2*64+k), col block j holds lag pair (2j, 2j+1)
    # wmat[l2*64+k, j*256+f] = W_pre[f, k, 31-(2j+l2)]
    wt = np.transpose(W_pre, (2, 1, 0))          # [L, K, F], wt[l,k,f]
    wmat = np.empty((128, NJ * F), dtype=np.float32)
    for j in range(NJ):
        for l2 in range(2):
            wmat[l2 * K:(l2 + 1) * K, j * F:(j + 1) * F] = wt[31 - (2 * j + l2)]

    import ml_dtypes
    bf16 = ml_dtypes.bfloat16
    # v3/v4 W layout: cols (c, jq, i, fo) from wmat cols j*F+f with
    # j = 2*jq+i, f = c*128+fo; each f-half c is a contiguous 2048-col block
    wmat3 = np.ascontiguousarray(
        wmat.reshape(128, 8, 2, 2, 128).transpose(0, 3, 1, 2, 4)
        .reshape(128, NJ * F).astype(bf16))

    in_maps = []
    for c in range(NCORES):
        t0 = c * TC
        # Hs_pre[k, j] = H_pre[k, T+30-t0-j], j in [0, HC)
        hrs = H_pre[:, T - TC - t0: T + L - 1 - t0][:, ::-1]  # [K, HC]
        hdup = np.empty((128, HC), dtype=np.float32)
        hdup[:K] = hrs
        hdup[K:, :HC - 1] = hrs[:, 1:]
        hdup[K:, HC - 1] = hrs[:, HC - 1]  # pad col, never read by matmuls
        dat = np.ascontiguousarray(data[:, t0:t0 + TC])
        # packed H: 64 unique rows x HC cols as [128, HPK] (two time-halves)
        hpk = np.zeros((128, HPK), dtype=np.float32)
        hpk[:K] = hrs[:, :HPK]
        hpk[K:, :HC - HPK] = hrs[:, HPK:]
        # v6 hybrid: duplicated lead [128, HLD] + packed tail [128, HPK2]
        hpk2 = np.zeros((128, HPK2), dtype=np.float32)
        hpk2[:K] = hrs[:, HLD:HLD + HPK2]
        hpk2[K:, :HC - HLD - HPK2] = hrs[:, HLD + HPK2:]
        in_maps.append({
            "hleadb": np.ascontiguousarray(hdup[:, :HLD].astype(bf16)),
            "hpk2b": np.ascontiguousarray(hpk2.astype(bf16)),
            "hdup": np.ascontiguousarray(hdup),
            "wmat": wmat,
            "dat": dat,
            "hdupb": np.ascontiguousarray(hdup.astype(bf16)),
            "wmatb": wmat3,
            "datb": np.ascontiguousarray(dat.astype(bf16)),
            "hpkb": np.ascontiguousarray(hpk.astype(bf16)),
        })
    return in_maps


_CACHED_NC = {}


def run_cores(data, W_pre, H_pre, mode="fp8", trace=False):
    if mode not in _CACHED_NC:
        _CACHED_NC[mode] = build_nc(mode)
    nc = _CACHED_NC[mode]
    in_maps = make_in_maps(data, W_pre, H_pre)
    res = bass_utils.run_bass_kernel_spmd(
        nc, in_maps, core_ids=list(range(NCORES)), trace=trace)
    return res


def kernel(data, W_pre, H_pre):
    res = run_cores(data, W_pre, H_pre, mode="v6", trace=False)
    sse = np.float64(0.0)
    for r in res.results:
        sse += r["out"].astype(np.float64).sum()
    return np.float32(sse / (F * T))

